# revision 3
# baseline (speedup 1.0000x reference)
"""Trainium2 Bass kernel for nn_Channel_M (noisy DNA channel simulator).

Strategy
--------
All randomness in the reference is input-independent (fixed jax key 42), so
the host precomputes, per channel c and row b:
  * dsub1 = substitution delta + 1 (int8 -> bf16)
  * tau   = scatter target for every data element (int16), -1 = dropped
  * syms  = inserted symbol values (+1) appended to the data row
The device then does, per 128-row tile and channel:
  1. y = (x + dsub1) - 4*(x + dsub1 >= 4.5)        (dense, DVE, bf16 exact)
  2. out_row = local_scatter(data=[y|syms], tau)    (gpsimd per-partition scatter;
     unwritten positions = 0 = pad, dropped elements = deletions/truncation)
  3. DMA out (bf16; host converts to f32)
Sharding: batch split across the 8 cores (embarrassingly parallel).
"""
import sys
import time

sys.path.insert(0, "/opt/trn_rl_repo")

import numpy as np

B, L = 16384, 512
LP = L + 2           # 514 output row length
NCH = 4
NCORES = 8
R = B // NCORES      # 2048 rows per core
TILE_P = 128
NTILES = R // TILE_P  # 16 tiles per core

LAST_EXEC_NS = None
LAST_TRACE = None


# ----------------------------------------------------------------- host model
def _jax_draws():
    import jax

    cpu = jax.devices("cpu")[0]
    key = jax.random.key(42)
    out = {}
    with jax.default_device(cpu):
        for c in range(NCH):
            kc = jax.random.fold_in(key, c)
            k1, k2, k3 = jax.random.split(kc, 3)
            k1a, k1b = jax.random.split(k1)
            k3a, k3b = jax.random.split(k3)
            out[c] = dict(
                sub_data=np.asarray(jax.random.randint(k1a, (B, L), 0, 4), dtype=np.int8),
                sub_u=np.asarray(jax.random.uniform(k1b, (B, L)), dtype=np.float32),
                del_u=np.asarray(jax.random.uniform(k2, (B, L)), dtype=np.float32),
                ins_u=np.asarray(jax.random.uniform(k3a, (B, LP)), dtype=np.float32),
                ins_sym=np.asarray(jax.random.randint(k3b, (B, LP), 0, 4), dtype=np.int8),
            )
    return out


def _build_channel_tau(dr, sub_error, del_error, ins_error, smax):
    """Scatter program for one channel: (dsub1 [B,L] i8, tau [B,L+smax] i16,
    syms [B,smax] i8)."""
    sub_mask = dr["sub_u"] < sub_error
    dsub1 = (np.where(sub_mask, dr["sub_data"], 0) + 1).astype(np.int8)

    keep = dr["del_u"] >= del_error
    pos = np.cumsum(keep, axis=1, dtype=np.int32) - 1
    lengths = keep.sum(axis=1).astype(np.int32)

    idx = np.arange(LP)[None, :]
    ins = (dr["ins_u"] < ins_error) & (idx < lengths[:, None])
    insi = ins.astype(np.int32)
    excl = np.cumsum(insi, axis=1) - insi

    p_clip = np.minimum(pos, LP - 1)
    tau_y = p_clip + np.take_along_axis(excl, p_clip, axis=1)
    tau_y = np.where(keep & (tau_y < LP), tau_y, -1).astype(np.int16)

    tau_s = np.full((B, smax), -1, dtype=np.int16)
    syms = np.zeros((B, smax), dtype=np.int8)
    b_i, j_i = np.nonzero(ins)
    if b_i.size:
        k_i = (np.cumsum(ins, axis=1) - 1)[b_i, j_i]
        assert k_i.max() < smax
        pos_ins = j_i + excl[b_i, j_i] + 1
        valid = pos_ins < LP
        tau_s[b_i[valid], k_i[valid]] = pos_ins[valid].astype(np.int16)
        syms[b_i, k_i] = dr["ins_sym"][b_i, j_i] + 1

    tau = np.concatenate([tau_y, tau_s], axis=1)
    return dsub1, tau, syms


# -------------------------------------------------------------- bass program
def _build_bass(ni, smax):
    import concourse.bacc as bacc
    import concourse.mybir as mybir

    BF16 = mybir.dt.bfloat16
    I16 = mybir.dt.int16
    ALU = mybir.AluOpType
    NBUF = 4

    nc = bacc.Bacc()
    x_ext = nc.declare_dram_parameter("x", [R, L], BF16, isOutput=False)
    d_ext = nc.declare_dram_parameter("dsub", [NCH, R, L], BF16, isOutput=False)
    s_ext = nc.declare_dram_parameter("syms", [NCH, R, smax], BF16, isOutput=False)
    t_ext = nc.declare_dram_parameter("tau", [NCH, R, ni], I16, isOutput=False)
    o_ext = nc.declare_dram_parameter("out", [NCH, R, LP], BF16, isOutput=True)

    xt = x_ext.rearrange("(n p) l -> n p l", p=TILE_P)
    dt_ = d_ext.rearrange("c (n p) l -> c n p l", p=TILE_P)
    st = s_ext.rearrange("c (n p) l -> c n p l", p=TILE_P)
    tt = t_ext.rearrange("c (n p) l -> c n p l", p=TILE_P)
    ot = o_ext.rearrange("c (n p) l -> c n p l", p=TILE_P)

    NIT = NTILES * NCH

    with (
        nc.sbuf_tensor([TILE_P, 2 * L], BF16) as xb,          # 2 x-tiles
        nc.sbuf_tensor([TILE_P, NBUF * L], BF16) as db,
        nc.sbuf_tensor([TILE_P, NBUF * ni], BF16) as datab,
        nc.sbuf_tensor([TILE_P, NBUF * ni], I16) as idxb,
        nc.sbuf_tensor([TILE_P, NBUF * LP], BF16) as outb,
        nc.sbuf_tensor([TILE_P, 2 * L], BF16) as tmpb,
        nc.semaphore("din") as din,       # input DMAs (inc 16 each)
        nc.semaphore("dve") as dve,       # dense compute done (inc 1/iter)
        nc.semaphore("lsc") as lsc,       # local_scatter done (inc 1/iter)
        nc.semaphore("dout") as dout,     # output DMAs (inc 16 each)
        nc.Block() as block,
    ):
        # per-iteration input DMA counts: c==0 iters load x too
        din_after = []  # cumulative inc after iteration i's loads
        tot = 0
        for i in range(NIT):
            tot += 16 * (4 if i % NCH == 0 else 3)
            din_after.append(tot)

        LAG = 2

        def emit_in(sync, i):
            t, c = divmod(i, NCH)
            sl = i % NBUF
            # WAR guards: buffer slot reused from iteration i-NBUF
            if i >= NBUF:
                sync.wait_ge(dve, i - NBUF + 1)        # db consumed
                sync.wait_ge(lsc, i - NBUF + 1)        # datab/idxb consumed
            if c == 0 and t >= 2:
                # x slot t%2 consumed by dve through iteration (t-2)*NCH+3
                sync.wait_ge(dve, (t - 2) * NCH + NCH)
            sync.dma_start(db[:, sl * L:(sl + 1) * L], dt_[c, t]).then_inc(din, 16)
            sync.dma_start(datab[:, sl * ni + L: sl * ni + L + smax],
                           st[c, t]).then_inc(din, 16)
            sync.dma_start(idxb[:, sl * ni:(sl + 1) * ni], tt[c, t]).then_inc(din, 16)
            if c == 0:
                sync.dma_start(xb[:, (t % 2) * L:(t % 2 + 1) * L], xt[t]).then_inc(din, 16)

        def emit_out(sync, i):
            t, c = divmod(i, NCH)
            sl = i % NBUF
            sync.wait_ge(lsc, i + 1)
            sync.dma_start(ot[c, t], outb[:, sl * LP:(sl + 1) * LP]).then_inc(dout, 16)

        @block.sync
        def _(sync):
            for i in range(NIT):
                emit_in(sync, i)
                if i >= LAG:
                    emit_out(sync, i - LAG)
            for i in range(NIT - LAG, NIT):
                emit_out(sync, i)

        @block.vector
        def _(vector):
            for i in range(NIT):
                t, c = divmod(i, NCH)
                sl = i % NBUF
                ts = i % 2
                vector.wait_ge(din, din_after[i])          # x, dsub loaded
                if i >= NBUF:
                    vector.wait_ge(lsc, i - NBUF + 1)      # datab slot free
                xs = xb[:, (t % 2) * L:(t % 2 + 1) * L]
                ds = db[:, sl * L:(sl + 1) * L]
                tmp = tmpb[:, ts * L:(ts + 1) * L]
                ys = datab[:, sl * ni: sl * ni + L]
                vector.tensor_add(ys, xs, ds)
                vector.tensor_scalar(tmp, ys, 4.5, None, op0=ALU.is_ge)
                vector.scalar_tensor_tensor(ys, tmp, -4.0, ys,
                                            op0=ALU.mult, op1=ALU.add).then_inc(dve, 1)

        @block.gpsimd
        def _(g):
            for i in range(NIT):
                sl = i % NBUF
                g.wait_ge(dve, i + 1)
                g.wait_ge(din, din_after[i])               # syms+tau loaded
                if i >= NBUF:
                    g.wait_ge(dout, 16 * (i - NBUF + 1))   # outb slot free
                g.local_scatter(
                    out_ap=outb[:, sl * LP:(sl + 1) * LP],
                    data_ap=datab[:, sl * ni:(sl + 1) * ni],
                    idxs_ap=idxb[:, sl * ni:(sl + 1) * ni],
                    channels=TILE_P,
                    num_elems=LP,
                    num_idxs=ni,
                ).then_inc(lsc, 1)

    nc.compile()
    return nc


def _install_ntff_hook():
    import types
    import antenv

    if "antenv.axon_hooks" in sys.modules:
        return True
    mod = types.ModuleType("antenv.axon_hooks")
    mod._hook = None
    mod.set_axon_ntff_profile_hook = lambda h: setattr(mod, "_hook", h)
    mod.get_axon_ntff_profile_hook = lambda: mod._hook
    sys.modules["antenv.axon_hooks"] = mod
    antenv.axon_hooks = mod
    try:
        from trn_agent_boot.trn_boot import _ntff_profile_via_ctypes
        mod._hook = _ntff_profile_via_ctypes("/opt/axon/libaxon_pjrt.so")
        return mod._hook is not None
    except Exception:
        return False


# -------------------------------------------------------------------- kernel
def kernel(segment_en, sub_error, del_error, ins_error):
    global LAST_EXEC_NS, LAST_TRACE
    import os
    from concourse.bass_utils import run_bass_kernel_spmd
    import concourse.mybir as mybir

    npbf16 = mybir.dt.np(mybir.dt.bfloat16)

    x = np.asarray(segment_en, dtype=np.float32)
    sub_e = float(np.asarray(sub_error))
    del_e = float(np.asarray(del_error))
    ins_e = float(np.asarray(ins_error))

    draws = _jax_draws()

    # determine smax (max insertions in any row) across channels
    smax = 2
    chans = []
    for c in range(NCH):
        dr = draws[c]
        ins = (dr["ins_u"] < ins_e)
        # upper bound on per-row insertions (before the length mask): cheap+safe
        m = int(ins.sum(axis=1).max()) if ins.size else 0
        smax = max(smax, m + 2)
    smax = (smax + 1) // 2 * 2
    ni = L + smax

    dsub_all = np.empty((NCH, B, L), dtype=np.int8)
    tau_all = np.empty((NCH, B, ni), dtype=np.int16)
    syms_all = np.empty((NCH, B, smax), dtype=np.int8)
    for c in range(NCH):
        d1, tau, syms = _build_channel_tau(draws[c], sub_e, del_e, ins_e, smax)
        dsub_all[c], tau_all[c], syms_all[c] = d1, tau, syms

    x_bf = x.astype(npbf16)
    dsub_bf = dsub_all.astype(npbf16)
    syms_bf = syms_all.astype(npbf16)

    nc = _build_bass(ni, smax)

    in_maps = []
    for core in range(NCORES):
        r0, r1 = core * R, (core + 1) * R
        in_maps.append({
            "x": x_bf[r0:r1],
            "dsub": dsub_bf[:, r0:r1],
            "syms": syms_bf[:, r0:r1],
            "tau": tau_all[:, r0:r1],
        })

    trace = bool(int(os.environ.get("KERNEL_TRACE", "0")))
    if trace:
        trace = _install_ntff_hook()
    res = run_bass_kernel_spmd(nc, in_maps, list(range(NCORES)), trace=trace)
    LAST_EXEC_NS = res.exec_time_ns
    LAST_TRACE = getattr(res, "instructions_and_trace", None)

    out = np.empty((B, NCH, LP), dtype=np.float32)
    for core in range(NCORES):
        r0 = core * R
        # device out layout [NCH, R, LP] -> [R, NCH, LP]
        out[r0:r0 + R] = np.transpose(
            res.results[core]["out"].astype(np.float32), (1, 0, 2))
    return out


if __name__ == "__main__":
    rng = np.random.default_rng(0)
    x = rng.integers(0, 4, size=(B, L)).astype(np.float32)
    t0 = time.time()
    out = kernel(x, 0.02, 0.01, 0.01)
    print("kernel wall:", time.time() - t0, "s; exec_ns:", LAST_EXEC_NS)
    print("out", out.shape, out.dtype)


# revision 4
# speedup vs baseline: 1.1291x; 1.1291x over previous
"""Trainium2 Bass kernel for nn_Channel_M (noisy DNA channel simulator).

Strategy
--------
All randomness in the reference is input-independent (fixed jax key 42), so
the host precomputes, per channel c and row b:
  * dsub1 = substitution delta + 1 (int8 -> bf16)
  * tau   = scatter target for every data element (int16), -1 = dropped
  * syms  = inserted symbol values (+1) appended to the data row
The device then does, per 128-row tile and channel:
  1. y = (x + dsub1) - 4*(x + dsub1 >= 4.5)        (dense, DVE, bf16 exact)
  2. out_row = local_scatter(data=[y|syms], tau)    (gpsimd per-partition scatter;
     unwritten positions = 0 = pad, dropped elements = deletions/truncation)
  3. DMA out (bf16; host converts to f32)
Sharding: batch split across the 8 cores (embarrassingly parallel).
"""
import sys
import time

sys.path.insert(0, "/opt/trn_rl_repo")

import numpy as np

B, L = 16384, 512
LP = L + 2           # 514 output row length
NCH = 4
NCORES = 8
R = B // NCORES      # 2048 rows per core
TILE_P = 128
NTILES = R // TILE_P  # 16 tiles per core

LAST_EXEC_NS = None
LAST_TRACE = None


# ----------------------------------------------------------------- host model
def _jax_draws():
    import jax

    cpu = jax.devices("cpu")[0]
    key = jax.random.key(42)
    out = {}
    with jax.default_device(cpu):
        for c in range(NCH):
            kc = jax.random.fold_in(key, c)
            k1, k2, k3 = jax.random.split(kc, 3)
            k1a, k1b = jax.random.split(k1)
            k3a, k3b = jax.random.split(k3)
            out[c] = dict(
                sub_data=np.asarray(jax.random.randint(k1a, (B, L), 0, 4), dtype=np.int8),
                sub_u=np.asarray(jax.random.uniform(k1b, (B, L)), dtype=np.float32),
                del_u=np.asarray(jax.random.uniform(k2, (B, L)), dtype=np.float32),
                ins_u=np.asarray(jax.random.uniform(k3a, (B, LP)), dtype=np.float32),
                ins_sym=np.asarray(jax.random.randint(k3b, (B, LP), 0, 4), dtype=np.int8),
            )
    return out


def _build_channel_tau(dr, sub_error, del_error, ins_error, smax):
    """Scatter program for one channel: (dsub1 [B,L] i8, tau [B,L+smax] i16,
    syms [B,smax] i8)."""
    sub_mask = dr["sub_u"] < sub_error
    dsub1 = (np.where(sub_mask, dr["sub_data"], 0) + 1).astype(np.int8)

    keep = dr["del_u"] >= del_error
    pos = np.cumsum(keep, axis=1, dtype=np.int32) - 1
    lengths = keep.sum(axis=1).astype(np.int32)

    idx = np.arange(LP)[None, :]
    ins = (dr["ins_u"] < ins_error) & (idx < lengths[:, None])
    insi = ins.astype(np.int32)
    excl = np.cumsum(insi, axis=1) - insi

    p_clip = np.minimum(pos, LP - 1)
    tau_y = p_clip + np.take_along_axis(excl, p_clip, axis=1)
    tau_y = np.where(keep & (tau_y < LP), tau_y, -1).astype(np.int16)

    tau_s = np.full((B, smax), -1, dtype=np.int16)
    syms = np.zeros((B, smax), dtype=np.int8)
    b_i, j_i = np.nonzero(ins)
    if b_i.size:
        k_i = (np.cumsum(ins, axis=1) - 1)[b_i, j_i]
        assert k_i.max() < smax
        pos_ins = j_i + excl[b_i, j_i] + 1
        valid = pos_ins < LP
        tau_s[b_i[valid], k_i[valid]] = pos_ins[valid].astype(np.int16)
        syms[b_i, k_i] = dr["ins_sym"][b_i, j_i] + 1

    tau = np.concatenate([tau_y, tau_s], axis=1)
    return dsub1, tau, syms


# -------------------------------------------------------------- bass program
def _build_bass(ni, smax):
    import concourse.bacc as bacc
    import concourse.mybir as mybir

    BF16 = mybir.dt.bfloat16
    I16 = mybir.dt.int16
    ALU = mybir.AluOpType
    NBUF = 4

    nc = bacc.Bacc()
    x_ext = nc.declare_dram_parameter("x", [R, L], BF16, isOutput=False)
    d_ext = nc.declare_dram_parameter("dsub", [NCH, R, L], BF16, isOutput=False)
    s_ext = nc.declare_dram_parameter("syms", [NCH, R, smax], BF16, isOutput=False)
    t_ext = nc.declare_dram_parameter("tau", [NCH, R, ni], I16, isOutput=False)
    o_ext = nc.declare_dram_parameter("out", [NCH, R, LP], BF16, isOutput=True)

    xt = x_ext.rearrange("(n p) l -> n p l", p=TILE_P)
    dt_ = d_ext.rearrange("c (n p) l -> c n p l", p=TILE_P)
    st = s_ext.rearrange("c (n p) l -> c n p l", p=TILE_P)
    tt = t_ext.rearrange("c (n p) l -> c n p l", p=TILE_P)
    ot = o_ext.rearrange("c (n p) l -> c n p l", p=TILE_P)

    NIT = NTILES * NCH

    with (
        nc.sbuf_tensor([TILE_P, 2 * L], BF16) as xb,          # 2 x-tiles
        nc.sbuf_tensor([TILE_P, NBUF * L], BF16) as db,
        nc.sbuf_tensor([TILE_P, NBUF * ni], BF16) as datab,
        nc.sbuf_tensor([TILE_P, NBUF * ni], I16) as idxb,
        nc.sbuf_tensor([TILE_P, NBUF * LP], BF16) as outb,
        nc.sbuf_tensor([TILE_P, 2 * L], BF16) as tmpb,
        nc.semaphore("din0") as din0,     # per-slot input-DMA sems (inc 48/use)
        nc.semaphore("din1") as din1,
        nc.semaphore("din2") as din2,
        nc.semaphore("din3") as din3,
        nc.semaphore("xs0") as xs0,       # x-tile sems (inc 16/use)
        nc.semaphore("xs1") as xs1,
        nc.semaphore("do0") as do0,       # per-slot output-DMA sems (inc 16/use)
        nc.semaphore("do1") as do1,
        nc.semaphore("do2") as do2,
        nc.semaphore("do3") as do3,
        nc.semaphore("dve") as dve,       # dense compute done (inc 1/iter)
        nc.semaphore("lsc") as lsc,       # local_scatter done (inc 1/iter)
        nc.Block() as block,
    ):
        din_s = [din0, din1, din2, din3]
        do_s = [do0, do1, do2, do3]
        xs_s = [xs0, xs1]
        LAG = 2

        # Per-slot cumulative thresholds are order-exact: a slot's next use
        # cannot even be ISSUED until its previous consumer finished (dve/lsc
        # waits below), so reaching 48*(uses) implies THIS use's 3 DMAs done.
        def emit_in(sync, i):
            t, c = divmod(i, NCH)
            sl = i % NBUF
            if i >= NBUF:
                sync.wait_ge(dve, i - NBUF + 1)        # db consumed
                sync.wait_ge(lsc, i - NBUF + 1)        # datab/idxb consumed
            if c == 0 and t >= 2:
                # x slot t%2 consumed by dve through iteration (t-2)*NCH+3
                sync.wait_ge(dve, (t - 2) * NCH + NCH)
            sync.dma_start(db[:, sl * L:(sl + 1) * L], dt_[c, t]).then_inc(din_s[sl], 16)
            sync.dma_start(datab[:, sl * ni + L: sl * ni + L + smax],
                           st[c, t]).then_inc(din_s[sl], 16)
            sync.dma_start(idxb[:, sl * ni:(sl + 1) * ni], tt[c, t]).then_inc(din_s[sl], 16)
            if c == 0:
                sync.dma_start(xb[:, (t % 2) * L:(t % 2 + 1) * L],
                               xt[t]).then_inc(xs_s[t % 2], 16)

        def emit_out(sync, i):
            t, c = divmod(i, NCH)
            sl = i % NBUF
            sync.wait_ge(lsc, i + 1)
            sync.dma_start(ot[c, t], outb[:, sl * LP:(sl + 1) * LP]).then_inc(do_s[sl], 16)

        @block.sync
        def _(sync):
            for i in range(NIT):
                emit_in(sync, i)
                if i >= LAG:
                    emit_out(sync, i - LAG)
            for i in range(NIT - LAG, NIT):
                emit_out(sync, i)

        @block.vector
        def _(vector):
            for i in range(NIT):
                t, c = divmod(i, NCH)
                sl = i % NBUF
                ts = i % 2
                vector.wait_ge(din_s[sl], 48 * (i // NBUF + 1))   # dsub in
                vector.wait_ge(xs_s[t % 2], 16 * (t // 2 + 1))    # x in
                if i >= NBUF:
                    vector.wait_ge(lsc, i - NBUF + 1)      # datab slot free
                xs = xb[:, (t % 2) * L:(t % 2 + 1) * L]
                ds = db[:, sl * L:(sl + 1) * L]
                tmp = tmpb[:, ts * L:(ts + 1) * L]
                ys = datab[:, sl * ni: sl * ni + L]
                vector.tensor_add(ys, xs, ds)
                vector.tensor_scalar(tmp, ys, 4.5, None, op0=ALU.is_ge)
                vector.scalar_tensor_tensor(ys, tmp, -4.0, ys,
                                            op0=ALU.mult, op1=ALU.add).then_inc(dve, 1)

        @block.gpsimd
        def _(g):
            for i in range(NIT):
                sl = i % NBUF
                g.wait_ge(dve, i + 1)
                g.wait_ge(din_s[sl], 48 * (i // NBUF + 1))  # syms+tau loaded
                if i >= NBUF:
                    g.wait_ge(do_s[sl], 16 * (i // NBUF))   # outb slot free
                g.local_scatter(
                    out_ap=outb[:, sl * LP:(sl + 1) * LP],
                    data_ap=datab[:, sl * ni:(sl + 1) * ni],
                    idxs_ap=idxb[:, sl * ni:(sl + 1) * ni],
                    channels=TILE_P,
                    num_elems=LP,
                    num_idxs=ni,
                ).then_inc(lsc, 1)

    nc.compile()
    return nc


def _install_ntff_hook():
    import types
    import antenv

    if "antenv.axon_hooks" in sys.modules:
        return True
    mod = types.ModuleType("antenv.axon_hooks")
    mod._hook = None
    mod.set_axon_ntff_profile_hook = lambda h: setattr(mod, "_hook", h)
    mod.get_axon_ntff_profile_hook = lambda: mod._hook
    sys.modules["antenv.axon_hooks"] = mod
    antenv.axon_hooks = mod
    try:
        from trn_agent_boot.trn_boot import _ntff_profile_via_ctypes
        mod._hook = _ntff_profile_via_ctypes("/opt/axon/libaxon_pjrt.so")
        return mod._hook is not None
    except Exception:
        return False


# -------------------------------------------------------------------- kernel
def kernel(segment_en, sub_error, del_error, ins_error):
    global LAST_EXEC_NS, LAST_TRACE
    import os
    from concourse.bass_utils import run_bass_kernel_spmd
    import concourse.mybir as mybir

    npbf16 = mybir.dt.np(mybir.dt.bfloat16)

    x = np.asarray(segment_en, dtype=np.float32)
    sub_e = float(np.asarray(sub_error))
    del_e = float(np.asarray(del_error))
    ins_e = float(np.asarray(ins_error))

    draws = _jax_draws()

    # determine smax (max insertions in any row) across channels
    smax = 2
    chans = []
    for c in range(NCH):
        dr = draws[c]
        ins = (dr["ins_u"] < ins_e)
        # upper bound on per-row insertions (before the length mask): cheap+safe
        m = int(ins.sum(axis=1).max()) if ins.size else 0
        smax = max(smax, m + 2)
    smax = (smax + 1) // 2 * 2
    ni = L + smax

    dsub_all = np.empty((NCH, B, L), dtype=np.int8)
    tau_all = np.empty((NCH, B, ni), dtype=np.int16)
    syms_all = np.empty((NCH, B, smax), dtype=np.int8)
    for c in range(NCH):
        d1, tau, syms = _build_channel_tau(draws[c], sub_e, del_e, ins_e, smax)
        dsub_all[c], tau_all[c], syms_all[c] = d1, tau, syms

    x_bf = x.astype(npbf16)
    dsub_bf = dsub_all.astype(npbf16)
    syms_bf = syms_all.astype(npbf16)

    nc = _build_bass(ni, smax)

    in_maps = []
    for core in range(NCORES):
        r0, r1 = core * R, (core + 1) * R
        in_maps.append({
            "x": x_bf[r0:r1],
            "dsub": dsub_bf[:, r0:r1],
            "syms": syms_bf[:, r0:r1],
            "tau": tau_all[:, r0:r1],
        })

    trace = bool(int(os.environ.get("KERNEL_TRACE", "0")))
    if trace:
        trace = _install_ntff_hook()
    res = run_bass_kernel_spmd(nc, in_maps, list(range(NCORES)), trace=trace)
    LAST_EXEC_NS = res.exec_time_ns
    LAST_TRACE = getattr(res, "instructions_and_trace", None)

    out = np.empty((B, NCH, LP), dtype=np.float32)
    for core in range(NCORES):
        r0 = core * R
        # device out layout [NCH, R, LP] -> [R, NCH, LP]
        out[r0:r0 + R] = np.transpose(
            res.results[core]["out"].astype(np.float32), (1, 0, 2))
    return out


if __name__ == "__main__":
    rng = np.random.default_rng(0)
    x = rng.integers(0, 4, size=(B, L)).astype(np.float32)
    t0 = time.time()
    out = kernel(x, 0.02, 0.01, 0.01)
    print("kernel wall:", time.time() - t0, "s; exec_ns:", LAST_EXEC_NS)
    print("out", out.shape, out.dtype)
